# revision 1
# baseline (speedup 1.0000x reference)
"""DirectedGraphConvolution Trainium2 kernel.

Per batch element b (one per NeuronCore, 8 total, data-parallel):
    N_e = H @ W                          [n, dout]
    T1  = G  @ N_e                       [n, dout]
    T2  = G.T @ N_e                      [n, dout]
    rs  = G.sum(-1); cs = G.sum(-2)
    out = [ relu(0.5*(T1 + T2)),             # G_F @ N_e,  G_F = (G+G.T)/2
            relu(G.T @ (T1 / rs[:,None])),   # G_Sin @ N_e
            relu(G  @ (T2 / cs[:,None])) ]   # G_So  @ N_e
(The reference's [n,n] Gram matrices are never materialized - associativity.)

Layouts: matmul computes out[m,n] = sum_p lhsT[p,m]*rhs[p,n].  G is kept
SBUF-resident in natural layout (row index on partitions), which directly
serves the G.T-left products (passes A and C).  G-left products (pass B)
get their stationary GT blocks from on-the-fly PE transposes, software-
pipelined twelve steps ahead of the consuming matmuls (batching the
transpose_mode instructions cuts PE mode-transition overhead).  cs falls
out of a ones-column fused into pass A's moving operand; rs is reduced
on DVE while G streams in.  Pass A: sweep 1 (jt 0-7) is it-outer across
8 PSUM banks so its matmuls track the G DMA arrival; sweep 2 (jt 8-15)
is jt-outer (G resident by then), which accumulates bank-local and
releases banks progressively so pass B's pipeline starts early.  All
matmuls run in float32r (fp32 storage, ~1 cyc/row at even N>=256).
"""

import numpy as np
import concourse.bass as bass
import concourse.mybir as mybir
import concourse.tile as tile
from concourse import bacc
from concourse.bass_utils import run_bass_kernel_spmd
from concourse.masks import make_identity

F32 = mybir.dt.float32
F32R = mybir.dt.float32r
RELU = mybir.ActivationFunctionType.Relu
AX = mybir.AxisListType.X

P = 128
B = 8
N = 2048
NO = N // P            # 16 row tiles
DIN = 256
DOUT = 256
KO = DIN // P          # 2 k tiles for H @ W
W3 = 3 * DOUT
RB = 2 + DOUT + DOUT   # rhs_b columns: [ones ones | N_e | T2'] (f32r needs even widths)


def build():
    nc = bacc.Bacc("TRN2", target_bir_lowering=False)
    G = nc.declare_dram_parameter("G", [N, N], F32, isOutput=False)
    H = nc.declare_dram_parameter("H", [N, DIN], F32, isOutput=False)
    W = nc.declare_dram_parameter("W", [DIN, DOUT], F32, isOutput=False)
    out = nc.declare_dram_parameter("out", [N, W3], F32, isOutput=True)

    G_r = G.rearrange("(o p) j -> p o j", p=P).bitcast(F32R)
    H_r = H.rearrange("(o p) d -> p o d", p=P).bitcast(F32R)
    W_r = W.rearrange("(o p) d -> p o d", p=P).bitcast(F32R)
    out_r = out.rearrange("(o p) d -> p o d", p=P)

    with tile.TileContext(nc) as tc:
        with (
            tc.tile_pool(name="const", bufs=1) as const,
            tc.tile_pool(name="gpool", bufs=1) as gpool,
            tc.tile_pool(name="big", bufs=1) as big,
            tc.tile_pool(name="hin", bufs=3) as hin,
            tc.tile_pool(name="stage", bufs=4) as stage,
            tc.tile_pool(name="gtp", bufs=12) as gtp,
            tc.tile_pool(name="tmpp", bufs=2) as tmpp,
        ):
            # G DMAs own the Sync HWDGE queue exclusively; everything else
            # (W, H, outputs) issues elsewhere so a slot-release wait can
            # never block the G stream behind it.
            g_tiles = [
                gpool.tile([P, N], F32R, tag=f"g{o}", name=f"g{o}")
                for o in range(NO)
            ]
            for o in range(NO):
                nc.sync.dma_start(g_tiles[o][:, 0:N // 2], G_r[:, o, 0:N // 2])
                nc.sync.dma_start(g_tiles[o][:, N // 2:N], G_r[:, o, N // 2:N])

            w_sb = const.tile([P, KO, DOUT], F32R)
            nc.scalar.dma_start(w_sb, W_r)

            ident_f32 = const.tile([P, P], F32)
            make_identity(nc, ident_f32)
            ident = const.tile([P, P], F32R)
            nc.vector.tensor_copy(ident, ident_f32)
            # rhsb[o] columns: [N_e | T2']
            rhsb = [
                big.tile([P, RB], F32R, tag=f"rb{o}", name=f"rb{o}") for o in range(NO)
            ]
            t1 = [
                big.tile([P, DOUT], F32R, tag=f"t1{o}", name=f"t1{o}")
                for o in range(NO)
            ]
            rsinv = const.tile([P, NO, 1], F32)
            ones_f32 = const.tile([P, 1], F32)
            nc.vector.memset(ones_f32, 1.0)
            cs_sb = const.tile([P, NO, 1], F32)
            csinv = const.tile([P, NO, 1], F32)
            for o in range(NO):
                nc.vector.tensor_copy(rhsb[o][:, 0:1], ones_f32)
                nc.vector.tensor_copy(rhsb[o][:, 1:2], ones_f32)

            # ---- N_e = H @ W  (transpose H blocks on PE, then matmul) ----
            with (
                tc.tile_pool(name="ps_ht", bufs=3, space="PSUM") as ps_ht,
                tc.tile_pool(name="ps_ne", bufs=2, space="PSUM") as ps_ne,
            ):
                # H tiles park in rhsb's T2' region (unused until pass A's
                # epilogue) so every H DMA issues immediately with no SBUF
                # slot-release wait -- a waiting DMA would block the shared
                # HWDGE semaphore slots the G stream cycles through.
                # software pipeline: transposes for tile t run while tile
                # t-1's matmuls consume the previous transposed block, so the
                # PE never stalls on the PSUM->SBUF copy between them
                for t in range(NO):
                    nc.scalar.dma_start(rhsb[t][:, 2 + DOUT:RB], H_r[:, t, :])
                hts = {}
                for t in range(NO + 1):
                    if t < NO:
                        h_t = rhsb[t][:, 2 + DOUT:RB]
                        ht_t = hin.tile([P, KO, P], F32R, tag="ht")
                        for kt in range(KO):
                            pt = ps_ht.tile([P, P], F32, tag="pht")
                            nc.tensor.transpose(
                                pt.bitcast(F32R), h_t[:, kt * P:(kt + 1) * P], ident
                            )
                            nc.vector.tensor_copy(ht_t[:, kt, :], pt.bitcast(F32R))
                        hts[t] = ht_t
                    if t >= 1:
                        u = t - 1
                        ht_u = hts.pop(u)
                        pne = ps_ne.tile([P, DOUT], F32, tag="pne")
                        for kt in range(KO):
                            nc.tensor.matmul(
                                pne,
                                ht_u[:, kt, :],
                                w_sb[:, kt, :],
                                start=(kt == 0),
                                stop=(kt == KO - 1),
                            )
                        nc.vector.tensor_copy(rhsb[u][:, 2:2 + DOUT], pne)

                # rs = row sums (DVE) as G tiles land
                for o in range(NO):
                    rs_t = tmpp.tile([P, 1], F32, tag="rs")
                    nc.vector.reduce_sum(rs_t, g_tiles[o].bitcast(F32), axis=AX)
                    nc.vector.reciprocal(rsinv[:, o, :], rs_t)

            # ---- pass A: [cs cs | T2] = G.T @ [ones ones | N_e] ----
            with tc.tile_pool(name="psA", bufs=8, space="PSUM") as psA:
                def a_epilogue(jt, pa):
                    nc.vector.tensor_copy(cs_sb[:, jt, :], pa[:, 0:1])
                    nc.vector.reciprocal(csinv[:, jt, :], pa[:, 0:1])
                    # T2' = T2 / cs  -> rhsb cols [2+DOUT : RB]
                    nc.vector.tensor_scalar_mul(
                        rhsb[jt][:, 2 + DOUT:RB],
                        pa[:, 2:2 + DOUT],
                        csinv[:, jt, 0:1],
                    )

                # sweep 1 (jt 0-7): it-outer across 8 banks, tracks G arrival
                pas = {
                    jt: psA.tile([P, 2 + DOUT], F32, tag="pa", name=f"pa{jt}")
                    for jt in range(8)
                }
                for it in range(NO):
                    for jt in range(8):
                        nc.tensor.matmul(
                            pas[jt],
                            g_tiles[it][:, jt * P:(jt + 1) * P],
                            rhsb[it][:, 0:2 + DOUT],
                            start=(it == 0),
                            stop=(it == NO - 1),
                        )
                for jt in range(8):
                    a_epilogue(jt, pas[jt])

                # sweep 2 (jt 8-15): G is resident by now, so go jt-outer --
                # consecutive matmuls accumulate into one bank (no per-matmul
                # bank cycling) and banks release progressively, letting pass
                # B's transpose pipeline claim PSUM early
                for jt in range(8, NO):
                    pa2 = psA.tile([P, 2 + DOUT], F32, tag="pa", name=f"pa{jt}")
                    for it in range(NO):
                        nc.tensor.matmul(
                            pa2,
                            g_tiles[it][:, jt * P:(jt + 1) * P],
                            rhsb[it][:, 0:2 + DOUT],
                            start=(it == 0),
                            stop=(it == NO - 1),
                        )
                    a_epilogue(jt, pa2)

            # ---- pass B: [T1 | out3raw] = G @ [N_e | T2'] ----
            # stationary GT blocks from PE transposes, pipelined ahead;
            # PSUM->SBUF block copies alternate DVE / ACT
            with (
                tc.tile_pool(name="psB", bufs=3, space="PSUM") as psB,
                tc.tile_pool(name="psT", bufs=5, space="PSUM") as psT,
            ):
                for it in range(NO):
                    pb = psB.tile([P, 2 * DOUT], F32, tag="pb")
                    gts = {}
                    LOOKAHEAD = 12
                    for step in range(NO + LOOKAHEAD):
                        if step < NO:
                            jt = step
                            pt = psT.tile([P, P], F32, tag="ptr")
                            nc.tensor.transpose(
                                pt.bitcast(F32R),
                                g_tiles[it][:, jt * P:(jt + 1) * P],
                                ident,
                            )
                            gt_t = gtp.tile([P, P], F32R, tag="gt")
                            if jt % 2 == 0:
                                nc.vector.tensor_copy(gt_t, pt.bitcast(F32R))
                            else:
                                nc.scalar.copy(gt_t, pt.bitcast(F32R))
                            gts[jt] = gt_t
                        if step >= LOOKAHEAD:
                            jt = step - LOOKAHEAD
                            nc.tensor.matmul(
                                pb,
                                gts.pop(jt),
                                rhsb[jt][:, 2:RB],
                                start=(jt == 0),
                                stop=(jt == NO - 1),
                            )
                    # out1 = relu(0.5*(T1 + cs*T2'))
                    t2r = tmpp.tile([P, DOUT], F32, tag="t2r")
                    nc.vector.tensor_scalar_mul(
                        t2r, rhsb[it][:, 2 + DOUT:RB].bitcast(F32), cs_sb[:, it, 0:1]
                    )
                    nc.vector.tensor_add(t2r, t2r, pb[:, 0:DOUT])
                    o1 = stage.tile([P, DOUT], F32, tag="o1")
                    nc.scalar.activation(o1, t2r, RELU, scale=0.5)
                    nc.sync.dma_start(out_r[:, it, 0:DOUT], o1)
                    # T1' = T1 / rs
                    nc.vector.tensor_scalar_mul(
                        t1[it], pb[:, 0:DOUT], rsinv[:, it, 0:1]
                    )
                    # out3 = relu(G @ T2')
                    o3 = stage.tile([P, DOUT], F32, tag="o3")
                    nc.scalar.activation(o3, pb[:, DOUT:2 * DOUT], RELU)
                    nc.sync.dma_start(out_r[:, it, 2 * DOUT:W3], o3)

            # ---- pass C: out2 = relu(G.T @ T1') ----
            with tc.tile_pool(name="psC", bufs=6, space="PSUM") as psC:
                for jt in range(NO):
                    pc = psC.tile([P, DOUT], F32, tag="pc")
                    for it in range(NO):
                        nc.tensor.matmul(
                            pc,
                            g_tiles[it][:, jt * P:(jt + 1) * P],
                            t1[it],
                            start=(it == 0),
                            stop=(it == NO - 1),
                        )
                    o2 = stage.tile([P, DOUT], F32, tag="o2")
                    nc.scalar.activation(o2, pc, RELU)
                    nc.sync.dma_start(out_r[:, jt, DOUT:2 * DOUT], o2)

    nc.compile()
    return nc


_NC = None


def _get_nc():
    global _NC
    if _NC is None:
        _NC = build()
    return _NC


def run(inputs: dict, trace: bool = False):
    """Run on 8 cores; returns (stacked_out [B,N,W3], BassKernelResults)."""
    H, G, W = inputs["H"], inputs["G"], inputs["W"]
    H = np.ascontiguousarray(H, dtype=np.float32)
    G = np.ascontiguousarray(G, dtype=np.float32)
    W = np.ascontiguousarray(W, dtype=np.float32)
    in_maps = [
        {"G": np.ascontiguousarray(G[b]), "H": np.ascontiguousarray(H[b]), "W": W}
        for b in range(B)
    ]
    nc = _get_nc()
    res = run_bass_kernel_spmd(nc, in_maps, core_ids=list(range(B)), trace=trace)
    out = np.stack([res.results[b]["out"] for b in range(B)], axis=0)
    return out, res


def kernel(H, G, W):
    out, _ = run({"H": H, "G": G, "W": W})
    return out

